# revision 1
# baseline (speedup 1.0000x reference)
"""RBF kernel matrix on 8 Trainium2 NeuronCores.

K[i, j] = exp(-gamma * ||x_i - y_j||^2),  x: (8192, 64), y: (8192, 64).

Strategy: shard rows of x across the 8 cores (1024 rows each), replicate y.
On the host we express -gamma*dist^2 as a single inner product of augmented
vectors, then split each factor into an fp16 hi/lo pair (22-bit effective
mantissa) so the PE can run at its fast 1-cycle/row fp16 rate instead of the
4x-slower fp32 path. The three needed cross products (hi*hi, lo*hi, hi*lo)
are stacked along the contraction dim:

    rows   0..63   xh_k  * yh_k      (feature hi*hi)
    rows  64..127  xl_k  * yh_k      (feature lo*hi)
    rows 128..191  xh_k  * yl_k      (feature hi*lo)
    rows 192..194  |x|^2 hi/lo pairs against -gamma hi/lo
    rows 195..196  1 * (-gamma*|y|^2) hi/lo

giving 197 rows total = one K=128 matmul + one K=69 matmul accumulating
into the same PSUM bank. PSUM then holds -gamma*dist^2 in fp32 (all fp16
products are exact in fp32), one ScalarE Exp activation per 4-bank PSUM
group finishes the tile, and each core DMAs its (1024, 8192) fp32 block
out in 1 MiB chunks.
"""

import numpy as np

from concourse import bacc, tile, mybir
from concourse.bass_utils import run_bass_kernel_spmd

N_CORES = 8
BX, BY, F = 8192, 8192, 64
M_CORE = BX // N_CORES      # 1024 rows of x per core
K1 = 128                    # rows in the first stacked matmul
K2 = 69                     # rows in the second (197 - 128)
MM_N = 512                  # one PSUM bank of fp32
GRP = 4                     # PSUM banks per activation group
GRP_N = MM_N * GRP          # 2048 columns per ACT/DMA tile

_cache: dict = {}


def _build():
    if "nc" in _cache:
        return _cache["nc"]

    f32 = mybir.dt.float32
    f16 = mybir.dt.float16
    nc = bacc.Bacc(None, target_bir_lowering=False, debug=False)
    x1 = nc.dram_tensor("x1", (K1, M_CORE), f16, kind="ExternalInput")
    x2 = nc.dram_tensor("x2", (K2, M_CORE), f16, kind="ExternalInput")
    y1 = nc.dram_tensor("y1", (K1, BY), f16, kind="ExternalInput")
    y2 = nc.dram_tensor("y2", (K2, BY), f16, kind="ExternalInput")
    out = nc.dram_tensor("out", (M_CORE, BY), f32, kind="ExternalOutput")

    with tile.TileContext(nc) as tc:
        with (
            tc.tile_pool(name="const", bufs=1) as cpool,
            tc.tile_pool(name="obuf", bufs=4) as opool,
            tc.tile_pool(name="psum", bufs=2, space="PSUM") as ppool,
        ):
            x1_sb = cpool.tile((K1, M_CORE), f16)
            x2_sb = cpool.tile((K2, M_CORE), f16)
            y1_sb = cpool.tile((K1, BY), f16)
            y2_sb = cpool.tile((K2, BY), f16)
            nc.sync.dma_start(out=x1_sb[:], in_=x1[:])
            nc.sync.dma_start(out=x2_sb[:], in_=x2[:])
            nc.sync.dma_start(out=y1_sb[:], in_=y1[:])
            nc.sync.dma_start(out=y2_sb[:], in_=y2[:])

            for mi in range(M_CORE // 128):          # 8 chunks of 128 rows
                w1 = x1_sb[:, mi * 128 : (mi + 1) * 128]
                w2 = x2_sb[:, mi * 128 : (mi + 1) * 128]
                for ni in range(BY // GRP_N):        # 4 groups of 2048 cols
                    ps = ppool.tile((128, GRP_N), f32)
                    # weight-major order: 4 banks with W1, then 4 with W2,
                    # so the PE reloads weights twice per group, not 8x.
                    for j in range(GRP):
                        c0 = ni * GRP_N + j * MM_N
                        nc.tensor.matmul(
                            ps[:, j * MM_N : (j + 1) * MM_N],
                            w1,
                            y1_sb[:, c0 : c0 + MM_N],
                            start=True,
                            stop=False,
                        )
                    for j in range(GRP):
                        c0 = ni * GRP_N + j * MM_N
                        nc.tensor.matmul(
                            ps[:, j * MM_N : (j + 1) * MM_N],
                            w2,
                            y2_sb[:, c0 : c0 + MM_N],
                            start=False,
                            stop=True,
                        )
                    ot = opool.tile((128, GRP_N), f32)
                    nc.scalar.activation(
                        ot[:], ps[:], mybir.ActivationFunctionType.Exp
                    )
                    nc.sync.dma_start(
                        out=out[
                            mi * 128 : (mi + 1) * 128,
                            ni * GRP_N : (ni + 1) * GRP_N,
                        ],
                        in_=ot[:],
                    )

    nc.compile()
    _cache["nc"] = nc
    return nc


def _split16(a):
    hi = a.astype(np.float16)
    lo = (a - hi.astype(np.float32)).astype(np.float16)
    return hi, lo


def _prep_inputs(x, y, gamma):
    x = np.ascontiguousarray(np.asarray(x, dtype=np.float32))
    y = np.ascontiguousarray(np.asarray(y, dtype=np.float32))
    g = np.float32(np.asarray(gamma, dtype=np.float32))

    xh, xl = _split16(x.T)                    # (64, 8192) each
    x_sq = (x.astype(np.float64) ** 2).sum(axis=1).astype(np.float32)
    xsh, xsl = _split16(x_sq[None, :])        # (1, 8192)

    yt = y.T * (2.0 * g)                      # fold 2*gamma into y features
    yh, yl = _split16(yt)                     # (64, 8192)
    y_sq = (y.astype(np.float64) ** 2).sum(axis=1).astype(np.float32)
    yq = (-g) * y_sq[None, :]
    yqh, yql = _split16(yq)                   # (1, 8192)
    gh, gl = _split16(np.full((1, BY), -g, dtype=np.float32))

    ones = np.ones((1, BY), dtype=np.float16)

    # x-side stacked rows (197, 8192) and matching y-side rows
    xs = np.concatenate(
        [xh, xl, xh, xsh, xsl, xsh, ones, ones], axis=0
    )  # 64+64+64+1+1+1+1+1 = 197
    ys = np.concatenate(
        [yh, yh, yl, gh, gh, gl, yqh, yql], axis=0
    )
    xs1, xs2 = xs[:K1], xs[K1:]
    ys1, ys2 = ys[:K1], ys[K1:]
    return xs1, xs2, np.ascontiguousarray(ys1), np.ascontiguousarray(ys2)


def _run(x, y, gamma, trace=False, tmpdir=None):
    nc = _build()
    xs1, xs2, ys1, ys2 = _prep_inputs(x, y, gamma)
    in_maps = [
        {
            "x1": np.ascontiguousarray(xs1[:, c * M_CORE : (c + 1) * M_CORE]),
            "x2": np.ascontiguousarray(xs2[:, c * M_CORE : (c + 1) * M_CORE]),
            "y1": ys1,
            "y2": ys2,
        }
        for c in range(N_CORES)
    ]
    res = run_bass_kernel_spmd(
        nc, in_maps, list(range(N_CORES)), trace=trace, tmpdir=tmpdir
    )
    full = np.concatenate([res.results[c]["out"] for c in range(N_CORES)], axis=0)
    return full, res


def kernel(x, y, gamma):
    full, _ = _run(x, y, gamma, trace=False)
    return full


def kernel_traced(x, y, gamma, tmpdir=None):
    """test.py helper: returns (output, BassKernelResults with profile)."""
    return _run(x, y, gamma, trace=True, tmpdir=tmpdir)



# revision 2
# speedup vs baseline: 1.6252x; 1.6252x over previous
"""RBF kernel matrix on 8 Trainium2 NeuronCores.

K[i, j] = exp(-gamma * ||x_i - y_j||^2),  x: (8192, 64), y: (8192, 64).

Strategy: shard rows of x across the 8 cores (1024 rows each), replicate y.

Numerics ("consistent rounding"): round x, y to fp16 ON THE HOST and compute
ALL terms of the expansion ||x-y||^2 = ||x||^2 + ||y||^2 - 2 x.y from the
SAME rounded vectors.  The device then computes exactly

    arg = 2*gamma * ( x_h . y_h  -  ||y_h||^2/2 )  -  gamma*||x_h||^2
        = -gamma * || x_h - y_h ||^2

i.e. the true RBF argument for the perturbed points (x_h, y_h).  The output
error is then  2*gamma*(dx - dy).(x - y), which vanishes exactly where the
kernel peaks (x ~ y), so a SINGLE fp16 matmul pass suffices (measured
rel_max ~ 6e-4 vs the 2e-2 gate).  Layout per core:

    rows  0..63   x_h^T (fp16)          vs  y_h^T (fp16)
    rows 64..65   ones                  vs  hi/lo fp16 of -||y_h||^2/2

one K=66 matmul per 512-col PSUM bank (single streaming pass - half the PE
work of a hi/lo scheme, and PE stays off the critical path even at the mid
p-state).  The ScalarE activation applies  Exp(psum*scale + bias)  with
scale = 2*gamma and bias = -gamma*||x_h||^2 as per-partition fp32 APs (both
runtime data - nothing about gamma is baked into the NEFF).  Output is
written bf16 (halves the HBM write traffic vs fp32; adds ~2e-3 rounding,
still far under the gate) and upcast to fp32 on the host.
"""

import numpy as np

from concourse import bacc, tile, mybir
from concourse.bass_utils import run_bass_kernel_spmd

N_CORES = 8
BX, BY, F = 8192, 8192, 64
M_CORE = BX // N_CORES      # 1024 rows of x per core
K = F + 2                   # 64 features + 2 rows for -||y||^2/2 hi/lo
MM_N = 512                  # one PSUM bank of fp32
GRP = 4                     # PSUM banks per ACT/DMA tile
GRP_N = MM_N * GRP          # 2048 columns per ACT/DMA tile
N_MI = M_CORE // 128        # 8 row chunks
N_NI = BY // GRP_N          # 4 column groups

_cache: dict = {}


def _build():
    if "nc" in _cache:
        return _cache["nc"]

    f32 = mybir.dt.float32
    f16 = mybir.dt.float16
    bf16 = mybir.dt.bfloat16
    nc = bacc.Bacc(None, target_bir_lowering=False, debug=False)
    xs = nc.dram_tensor("xs", (K, M_CORE), f16, kind="ExternalInput")
    ys = nc.dram_tensor("ys", (K, BY), f16, kind="ExternalInput")
    xq = nc.dram_tensor("xq", (128, N_MI), f32, kind="ExternalInput")
    gs = nc.dram_tensor("gs", (128, 1), f32, kind="ExternalInput")
    out = nc.dram_tensor("out", (M_CORE, BY), bf16, kind="ExternalOutput")

    with tile.TileContext(nc) as tc:
        with (
            tc.tile_pool(name="const", bufs=1) as cpool,
            tc.tile_pool(name="obuf", bufs=4) as opool,
            tc.tile_pool(name="psum", bufs=2, space="PSUM") as ppool,
        ):
            xs_sb = cpool.tile((K, M_CORE), f16)
            ys_sb = cpool.tile((K, BY), f16)
            xq_sb = cpool.tile((128, N_MI), f32)
            gs_sb = cpool.tile((128, 1), f32)
            nc.sync.dma_start(out=xq_sb[:], in_=xq[:])
            nc.sync.dma_start(out=gs_sb[:], in_=gs[:])
            nc.sync.dma_start(out=xs_sb[:], in_=xs[:])
            # chunked y load so the first matmuls start early
            for q in range(N_NI):
                nc.sync.dma_start(
                    out=ys_sb[:, q * GRP_N : (q + 1) * GRP_N],
                    in_=ys[:, q * GRP_N : (q + 1) * GRP_N],
                )

            for mi in range(N_MI):
                w = xs_sb[:, mi * 128 : (mi + 1) * 128]
                for ni in range(N_NI):
                    ps = ppool.tile((128, GRP_N), f32)
                    for j in range(GRP):
                        c0 = ni * GRP_N + j * MM_N
                        nc.tensor.matmul(
                            ps[:, j * MM_N : (j + 1) * MM_N],
                            w,
                            ys_sb[:, c0 : c0 + MM_N],
                            start=True,
                            stop=True,
                        )
                    ot = opool.tile((128, GRP_N), bf16)
                    nc.scalar.activation(
                        ot[:],
                        ps[:],
                        mybir.ActivationFunctionType.Exp,
                        bias=xq_sb[:, mi : mi + 1],
                        scale=gs_sb[:, 0:1],
                    )
                    nc.sync.dma_start(
                        out=out[
                            mi * 128 : (mi + 1) * 128,
                            ni * GRP_N : (ni + 1) * GRP_N,
                        ],
                        in_=ot[:],
                    )

    nc.compile()
    _cache["nc"] = nc
    return nc


def _prep_inputs(x, y, gamma):
    x = np.ascontiguousarray(np.asarray(x, dtype=np.float32))
    y = np.ascontiguousarray(np.asarray(y, dtype=np.float32))
    g = np.float64(np.asarray(gamma, dtype=np.float32))

    xh = x.astype(np.float16)                       # rounded x
    yh = y.astype(np.float16)                       # rounded y
    xsq = (xh.astype(np.float64) ** 2).sum(axis=1)  # ||x_h||^2 (exact-ish)
    ysq = (yh.astype(np.float64) ** 2).sum(axis=1)

    ones = np.ones((2, BY), dtype=np.float16)
    yqv = -0.5 * ysq                                # scale 2*gamma applied later
    yq1 = yqv.astype(np.float16)
    yq2 = (yqv - yq1.astype(np.float64)).astype(np.float16)
    ys_full = np.concatenate([yh.T, yq1[None, :], yq2[None, :]], axis=0)
    ys_full = np.ascontiguousarray(ys_full)         # (66, 8192) fp16

    xq_full = (-g * xsq).astype(np.float32)         # (8192,) bias rows
    gs = np.full((128, 1), 2.0 * g, dtype=np.float32)

    xs_cores, xq_cores = [], []
    for c in range(N_CORES):
        sl = slice(c * M_CORE, (c + 1) * M_CORE)
        xs_c = np.concatenate([xh[sl].T, ones[:, :M_CORE]], axis=0)
        xs_cores.append(np.ascontiguousarray(xs_c))             # (66, 1024)
        xq_c = xq_full[sl].reshape(N_MI, 128).T                 # (128, 8)
        xq_cores.append(np.ascontiguousarray(xq_c))
    return xs_cores, ys_full, xq_cores, gs


def _run(x, y, gamma, trace=False, tmpdir=None):
    nc = _build()
    xs_cores, ys_full, xq_cores, gs = _prep_inputs(x, y, gamma)
    in_maps = [
        {"xs": xs_cores[c], "ys": ys_full, "xq": xq_cores[c], "gs": gs}
        for c in range(N_CORES)
    ]
    res = run_bass_kernel_spmd(
        nc, in_maps, list(range(N_CORES)), trace=trace, tmpdir=tmpdir
    )
    full = np.concatenate(
        [np.asarray(res.results[c]["out"]) for c in range(N_CORES)], axis=0
    )
    return full.astype(np.float32), res


def kernel(x, y, gamma):
    full, _ = _run(x, y, gamma, trace=False)
    return full


def kernel_traced(x, y, gamma, tmpdir=None):
    """test.py helper: returns (output, BassKernelResults with profile)."""
    return _run(x, y, gamma, trace=True, tmpdir=tmpdir)
